# revision 39
# baseline (speedup 1.0000x reference)
"""Gaussian-mixture log-likelihood kernel for 8 Trainium2 NeuronCores.

Math: ll_i = logsumexp_j( -0.5 x_i^T A_j x_i + x_i^T m_j + bias_j ) - C
with A_j = S_j S_j^T.  The quadratic form is a single contraction of 564
"lift" rows per point against a [564, K] parameter matrix: 496 unique
symmetric-pair products, 32 squares, 32 linear rows, 3 bias rows (split so
fp8 quantization of the bias is exact to ~1e-3) and 1 zero pad.  Lift rows
and parameters are stored in fp8-e4m3 with per-row-type power-of-two scales
folded oppositely into the two factors, so the PE runs DoubleRow matmuls
(256-row contraction per instruction, 2x column rate).

Layout is K-on-partitions: the parameter matrix is the stationary operand
and scores land as [K-half, points] in PSUM, 6 matmuls per 256-point chunk.
One [128, 2048] Exp turns a 4-chunk PSUM slab into f16 likelihoods; the sum
over K is then a second matmul pass (e-tile stationary x a ones column,
output free size 1), which costs the PE almost nothing and keeps the sum in
f32.  A global shift C (folded into the bias rows) makes exp() safe without
a per-point max; one Ln + bias-add finishes all 16384 points per core.

The lift planes are packed host-side (cheap: O(N D^2) vs the device's
O(N K D^2) contraction) and shipped as fp8, which also removes all
shuffle/multiply traffic from DVE/Pool.

Sharding: data-parallel over points, 16384 points/core; K-sized parameters
are replicated (precomputed on host in float64 -- tiny vs the N*K work).
"""

import sys

sys.path.insert(0, "/opt/trn_rl_repo")

import numpy as np
import ml_dtypes

import concourse.bass as bass
import concourse.bacc as bacc
import concourse.mybir as mybir
from concourse import bass_utils
from concourse.bass_interp import get_hw_module
from concourse.tile import TileContext

N, K, D = 131072, 256, 32
NCORES = 8
NC_PTS = N // NCORES            # 16384 points per core
NTILES = NC_PTS // 128          # 128 output columns
DSLAB = 1024                    # points per DMA slab
NDMA = NC_PTS // DSLAB          # 16
PSLAB = 1024                    # points per PSUM slab (4 chunks x 2 K-halves)
CHUNK = 256                     # points per matmul chunk
F32 = mybir.dt.float32
F16 = mybir.dt.float16
F8 = mybir.dt.float8e4

NROWS = 564                     # 512 (chunks A,B) + 52 (chunk C)
CROWS = 26                      # chunk C pair rows

SCL_P = 16.0                    # off-diagonal product rows
SCL_D = 4.0                     # diagonal (square) rows
SCL_L = 4.0                     # linear rows

_CACHE = {}


def _build(nc):
    lab = nc.dram_tensor("lab", [128, 4, NC_PTS], F8, kind="ExternalInput").ap()
    lc = nc.dram_tensor("lc", [CROWS, 2, NC_PTS], F8, kind="ExternalInput").ap()
    bab = nc.dram_tensor("bab", [128, 4, K], F8, kind="ExternalInput").ap()
    bc = nc.dram_tensor("bc", [CROWS, 2, K], F8, kind="ExternalInput").ap()
    out = nc.dram_tensor("out", [128, NTILES], F32, kind="ExternalOutput").ap()

    DR = mybir.MatmulPerfMode.DoubleRow

    with TileContext(nc) as tc:
        with (
            tc.tile_pool(name="par", bufs=1) as par_pool,
            tc.tile_pool(name="src", bufs=4) as src_pool,
            tc.tile_pool(name="eps", bufs=4) as eps_pool,
            tc.tile_pool(name="acc", bufs=1) as acc_pool,
            tc.tile_pool(name="psum", bufs=2, space="PSUM") as psum_pool,
        ):
            # split param issue across the SP and ACT DGE queues so the lift
            # stream and the (tiny) params overlap at startup.
            bab_t = par_pool.tile([128, 4, K], F8, tag="bab")
            bc_t = par_pool.tile([CROWS, 2, K], F8, tag="bc")
            ones = par_pool.tile([128, 1], F16, tag="ones")
            warm = par_pool.tile([128, 512], F16, tag="warm")
            nc.sync.dma_start(out=bab_t[:, :, :], in_=bab[:, :, :])
            nc.scalar.dma_start(out=bc_t[:, :, :], in_=bc[:, :, :])
            nc.gpsimd.memset(ones[:, :], 1.0)
            nc.gpsimd.memset(warm[:, :], 0.0)

            s_all = acc_pool.tile([128, NTILES], F32, tag="s_all")

            # ramp the PE p-state on junk during the initial DMA window so
            # the first real matmuls run at (nearly) full clock
            ps_warm = psum_pool.tile([128, 4, 2, CHUNK], F32, tag="ps")
            for _ in range(3):
                nc.tensor.matmul(out=ps_warm[0:1, 0, :, :],
                                 lhsT=warm[:, 0:1], rhs=warm[:, :],
                                 start=True, stop=True)

            def emit_sums(ps, e_t, col, ntiles):
                # sum over K: e-tile stationary x ones column -> [128, 1] per
                # point-tile, accumulated into columns of bank 0 of the (now
                # dead) score psum.  Emitted one slab late so the in-order PE
                # stream never stalls on the Exp of its own slab.
                for u in range(ntiles):
                    c, w = u // 2, u % 2
                    sl = slice(128 * w, 128 * (w + 1))
                    for h in range(2):
                        nc.tensor.matmul(out=ps[:, 0, 0, u:u + 1],
                                         lhsT=e_t[:, c, h, sl],
                                         rhs=ones[:, :],
                                         start=(u == 0 and h == 0),
                                         stop=(u == ntiles - 1 and h == 1))
                nc.vector.tensor_copy(out=s_all[:, col:col + ntiles],
                                      in_=ps[:, 0, 0, 0:ntiles])

            # 512-point slabs at both ends shorten the pipeline fill (first
            # Exp waits on half the data) and drain; 1024-point slabs in the
            # middle amortize the ACT per-instruction overhead.
            sizes = [1024] * 16
            pending = None
            lo = 0
            for npts in sizes:
                dlo = lo - lo % DSLAB
                if lo == dlo:  # first slab touching this DMA tile loads it
                    lab_t = src_pool.tile([128, 4, DSLAB], F8, tag="lab")
                    lc_t = src_pool.tile([CROWS, 2, DSLAB], F8, tag="lc")
                    if False:
                        pass
                    else:
                        nc.sync.dma_start(out=lab_t[:, :, :],
                                          in_=lab[:, :, dlo:dlo + DSLAB])
                        nc.gpsimd.dma_start(out=lc_t[:, :, :],
                                            in_=lc[:, :, dlo:dlo + DSLAB])

                nch = npts // CHUNK
                ps = psum_pool.tile([128, 4, 2, CHUNK], F32, tag="ps")
                for c in range(nch):
                    cs = slice(lo - dlo + c * CHUNK, lo - dlo + (c + 1) * CHUNK)
                    for h in range(2):
                        ks = slice(128 * h, 128 * (h + 1))
                        nc.tensor.matmul(out=ps[:, c, h, :],
                                         lhsT=bab_t[:, 0:2, ks],
                                         rhs=lab_t[:, 0:2, cs],
                                         start=(h == 0), stop=False,
                                         perf_mode=DR)
                        nc.tensor.matmul(out=ps[:, c, h, :],
                                         lhsT=bab_t[:, 2:4, ks],
                                         rhs=lab_t[:, 2:4, cs],
                                         start=False, stop=False,
                                         perf_mode=DR)
                        nc.tensor.matmul(out=ps[:, c, h, :],
                                         lhsT=bc_t[:, :, ks],
                                         rhs=lc_t[:, :, cs],
                                         start=False, stop=(h == 1),
                                         perf_mode=DR)
                e_t = eps_pool.tile([128, 4, 2, CHUNK], F16, tag="e")
                nc.scalar.activation(out=e_t[:, 0:nch, :, :],
                                     in_=ps[:, 0:nch, :, :],
                                     func=mybir.ActivationFunctionType.Exp)
                if pending is not None:
                    emit_sums(*pending)
                    pc = pending[2] + pending[3]
                    if pc == 64:
                        nc.scalar.dma_start(out=out[:, 0:64],
                                            in_=s_all[:, 0:64])
                    elif pc == 112:
                        nc.scalar.dma_start(out=out[:, 64:112],
                                            in_=s_all[:, 64:112])
                pending = (ps, e_t, lo // 128, npts // 128)
                lo += npts
            emit_sums(*pending)
            # the final ln(s) - C runs on host; ship the sums
            nc.sync.dma_start(out=out[:, 112:], in_=s_all[:, 112:])
    return nc


def _get_module():
    if "nc" not in _CACHE:
        nc = bacc.Bacc("TRN2", target_bir_lowering=False, debug=False,
                       num_devices=NCORES)
        _build(nc)
        # During the act-table placement pass only, hide every func set except
        # natural_log_exp_and_others (its real act_info index is preserved),
        # so Exp and Ln share one table load instead of reloading before the
        # final Ln.  Restored immediately -- runtime sees the true tables.
        import concourse.hw_specs as _hw_specs
        _orig = _hw_specs.get_activation_tables

        def _patched(arch):
            return {name: (funcs if name == "natural_log_exp_and_others"
                           else set())
                    for name, funcs in _orig(arch).items()}

        if False:  # runtime rejects set 6 (NRT_EXEC_UNIT_UNRECOVERABLE)
            _hw_specs.get_activation_tables = _patched
            bacc.get_activation_tables = _patched
        try:
            nc.compile()
        finally:
            _hw_specs.get_activation_tables = _orig
            bacc.get_activation_tables = _orig
        nc.m = get_hw_module(nc.m)
        _CACHE["nc"] = nc
    return _CACHE["nc"]


def _host_params(centers, covs_inv_sqrt, weights, threshold):
    """Per-cluster parameter rows (B) scaled for fp8, plus exp(-C)."""
    S = covs_inv_sqrt.astype(np.float64)
    w = np.abs(weights.astype(np.float64))
    cp = w / (w.sum() + 1e-30)
    A = np.einsum("kde,kfe->kdf", S, S)
    _, logdetS = np.linalg.slogdet(S)
    logcoef = np.log(np.maximum(cp, 1e-300)) + logdetS
    cen = centers.astype(np.float64)
    m = np.einsum("kde,ke->kd", A, cen)
    t_cAc = np.einsum("kd,kd->k", m, cen)
    thr = float(threshold[0])
    C = 4.0 - (logcoef.max() - thr)
    bias = logcoef - 0.5 * t_cAc - thr + C

    Brows = np.zeros((NROWS, K), np.float64)
    r = 0
    for o in range(1, 16):                      # 480 off-diag product rows
        for i in range(32):
            Brows[r] = -A[:, i, (i + o) % 32] * SCL_P
            r += 1
    for i in range(16):                         # 16 distance-16 pairs
        Brows[r] = -A[:, i, i + 16] * SCL_P
        r += 1
    for i in range(32):                         # squares
        Brows[r] = -0.5 * A[:, i, i] * SCL_D
        r += 1
    for i in range(32):                         # linear
        Brows[r] = m[:, i] * SCL_L
        r += 1
    f8 = ml_dtypes.float8_e4m3
    b0 = bias.astype(f8).astype(np.float64)
    b1 = (bias - b0).astype(f8).astype(np.float64)
    Brows[r] = b0
    Brows[r + 1] = b1
    Brows[r + 2] = bias - b0 - b1
    # row r+3 stays zero (pad)
    return Brows.astype(np.float32), np.float64(C)


def _host_lift(pts):
    """fp8-ready lift planes [NROWS, npts] for one core's points [npts, 32]."""
    X = np.ascontiguousarray(pts.T)             # [32, npts]
    npts = X.shape[1]
    L = np.empty((NROWS, npts), np.float32)
    r = 0
    inv_p = np.float32(1.0 / SCL_P)
    for o in range(1, 16):
        L[r:r + 32] = X * np.roll(X, -o, axis=0) * inv_p
        r += 32
    L[r:r + 16] = X[:16] * X[16:] * inv_p
    r += 16
    L[r:r + 32] = X * X * np.float32(1.0 / SCL_D)
    r += 32
    L[r:r + 32] = X * np.float32(1.0 / SCL_L)
    r += 32
    L[r:r + 2] = 1.0
    L[r + 2] = 1.0
    L[r + 3] = 0.0
    return L


def kernel(points, centers, covs_inv_sqrt, weights, threshold):
    points = np.asarray(points, dtype=np.float32)
    Brows, C = _host_params(np.asarray(centers),
                            np.asarray(covs_inv_sqrt),
                            np.asarray(weights), np.asarray(threshold))
    f8 = ml_dtypes.float8_e4m3
    B8 = Brows.astype(f8)
    bab = np.ascontiguousarray(
        B8[:512].reshape(4, 128, K).transpose(1, 0, 2))
    bc = np.ascontiguousarray(
        B8[512:].reshape(2, CROWS, K).transpose(1, 0, 2))
    in_maps = []
    for r in range(NCORES):
        L8 = _host_lift(points[r * NC_PTS:(r + 1) * NC_PTS]).astype(f8)
        lab = np.ascontiguousarray(
            L8[:512].reshape(4, 128, NC_PTS).transpose(1, 0, 2))
        lc = np.ascontiguousarray(
            L8[512:].reshape(2, CROWS, NC_PTS).transpose(1, 0, 2))
        in_maps.append({"lab": lab, "lc": lc, "bab": bab, "bc": bc})

    nc = _get_module()
    res = bass_utils.run_bass_kernel_spmd(nc, in_maps,
                                          core_ids=list(range(NCORES)))
    sv = np.concatenate([res.results[r]["out"].T.reshape(-1)
                         for r in range(NCORES)])
    ll = np.log(sv.astype(np.float64)) - C
    return ll.reshape(N, 1).astype(np.float32)


# revision 40
# speedup vs baseline: 1.0101x; 1.0101x over previous
"""Gaussian-mixture log-likelihood kernel for 8 Trainium2 NeuronCores.

Math: ll_i = logsumexp_j( -0.5 x_i^T A_j x_i + x_i^T m_j + bias_j ) - C
with A_j = S_j S_j^T.  The quadratic form is a single contraction of 564
"lift" rows per point against a [564, K] parameter matrix: 496 unique
symmetric-pair products, 32 squares, 32 linear rows, 3 bias rows (split so
fp8 quantization of the bias is exact to ~1e-3) and 1 zero pad.  Lift rows
and parameters are stored in fp8-e4m3 with per-row-type power-of-two scales
folded oppositely into the two factors, so the PE runs DoubleRow matmuls
(256-row contraction per instruction, 2x column rate).

Layout is K-on-partitions: the parameter matrix is the stationary operand
and scores land as [K-half, points] in PSUM, 6 matmuls per 256-point chunk.
One [128, 2048] Exp turns a 4-chunk PSUM slab into f16 likelihoods; the sum
over K is then a second matmul pass (e-tile stationary x a ones column,
output free size 1), which costs the PE almost nothing and keeps the sum in
f32.  A global shift C (folded into the bias rows) makes exp() safe without
a per-point max; one Ln + bias-add finishes all 16384 points per core.

The lift planes are packed host-side (cheap: O(N D^2) vs the device's
O(N K D^2) contraction) and shipped as fp8, which also removes all
shuffle/multiply traffic from DVE/Pool.

Sharding: data-parallel over points, 16384 points/core; K-sized parameters
are replicated (precomputed on host in float64 -- tiny vs the N*K work).
"""

import sys

sys.path.insert(0, "/opt/trn_rl_repo")

import numpy as np
import ml_dtypes

import concourse.bass as bass
import concourse.bacc as bacc
import concourse.mybir as mybir
from concourse import bass_utils
from concourse.bass_interp import get_hw_module
from concourse.tile import TileContext

N, K, D = 131072, 256, 32
NCORES = 8
NC_PTS = N // NCORES            # 16384 points per core
NTILES = NC_PTS // 128          # 128 output columns
DSLAB = 1024                    # points per DMA slab
NDMA = NC_PTS // DSLAB          # 16
PSLAB = 1024                    # points per PSUM slab (4 chunks x 2 K-halves)
CHUNK = 256                     # points per matmul chunk
F32 = mybir.dt.float32
F16 = mybir.dt.float16
F8 = mybir.dt.float8e4

NROWS = 564                     # 512 (chunks A,B) + 52 (chunk C)
CROWS = 26                      # chunk C pair rows

SCL_P = 16.0                    # off-diagonal product rows
SCL_D = 4.0                     # diagonal (square) rows
SCL_L = 4.0                     # linear rows

_CACHE = {}


def _build(nc):
    lab = nc.dram_tensor("lab", [128, 4, NC_PTS], F8, kind="ExternalInput").ap()
    lc = nc.dram_tensor("lc", [CROWS, 2, NC_PTS], F8, kind="ExternalInput").ap()
    bab = nc.dram_tensor("bab", [128, 4, K], F8, kind="ExternalInput").ap()
    bc = nc.dram_tensor("bc", [CROWS, 2, K], F8, kind="ExternalInput").ap()
    out = nc.dram_tensor("out", [128, NTILES], F32, kind="ExternalOutput").ap()

    DR = mybir.MatmulPerfMode.DoubleRow

    with TileContext(nc) as tc:
        with (
            tc.tile_pool(name="par", bufs=1) as par_pool,
            tc.tile_pool(name="src", bufs=4) as src_pool,
            tc.tile_pool(name="eps", bufs=4) as eps_pool,
            tc.tile_pool(name="acc", bufs=1) as acc_pool,
            tc.tile_pool(name="psum", bufs=2, space="PSUM") as psum_pool,
        ):
            # split param issue across the SP and ACT DGE queues so the lift
            # stream and the (tiny) params overlap at startup.
            bab_t = par_pool.tile([128, 4, K], F8, tag="bab")
            bc_t = par_pool.tile([CROWS, 2, K], F8, tag="bc")
            ones = par_pool.tile([128, 1], F16, tag="ones")
            warm = par_pool.tile([128, 512], F16, tag="warm")
            nc.sync.dma_start(out=bab_t[:, :, :], in_=bab[:, :, :])
            nc.scalar.dma_start(out=bc_t[:, :, :], in_=bc[:, :, :])
            nc.gpsimd.memset(ones[:, :], 1.0)
            nc.gpsimd.memset(warm[:, :], 0.0)

            s_all = acc_pool.tile([128, NTILES], F32, tag="s_all")

            # ramp the PE p-state on junk during the initial DMA window so
            # the first real matmuls run at (nearly) full clock
            ps_warm = psum_pool.tile([128, 4, 2, CHUNK], F32, tag="ps")
            for _ in range(3):
                nc.tensor.matmul(out=ps_warm[0:1, 0, :, :],
                                 lhsT=warm[:, 0:1], rhs=warm[:, :],
                                 start=True, stop=True)

            def emit_sums(ps, e_t, col, ntiles):
                # sum over K: e-tile stationary x ones column -> [128, 1] per
                # point-tile, accumulated into columns of bank 0 of the (now
                # dead) score psum.  Emitted one slab late so the in-order PE
                # stream never stalls on the Exp of its own slab.
                for u in range(ntiles):
                    c, w = u // 2, u % 2
                    sl = slice(128 * w, 128 * (w + 1))
                    for h in range(2):
                        nc.tensor.matmul(out=ps[:, 0, 0, u:u + 1],
                                         lhsT=e_t[:, c, h, sl],
                                         rhs=ones[:, :],
                                         start=(u == 0 and h == 0),
                                         stop=(u == ntiles - 1 and h == 1))
                nc.vector.tensor_copy(out=s_all[:, col:col + ntiles],
                                      in_=ps[:, 0, 0, 0:ntiles])

            # 512-point slabs at both ends shorten the pipeline fill (first
            # Exp waits on half the data) and drain; 1024-point slabs in the
            # middle amortize the ACT per-instruction overhead.
            sizes = [1024] * 16
            pending = None
            lo = 0
            for npts in sizes:
                dlo = lo - lo % DSLAB
                if lo == dlo:  # first slab touching this DMA tile loads it
                    lab_t = src_pool.tile([128, 4, DSLAB], F8, tag="lab")
                    lc_t = src_pool.tile([CROWS, 2, DSLAB], F8, tag="lc")
                    if lo == 0:
                        nc.sync.dma_start(out=lab_t[:, :, 0:512],
                                          in_=lab[:, :, 0:512])
                        nc.gpsimd.dma_start(out=lc_t[:, :, :],
                                            in_=lc[:, :, 0:DSLAB])
                        nc.sync.dma_start(out=lab_t[:, :, 512:DSLAB],
                                          in_=lab[:, :, 512:DSLAB])
                    else:
                        nc.sync.dma_start(out=lab_t[:, :, :],
                                          in_=lab[:, :, dlo:dlo + DSLAB])
                        nc.gpsimd.dma_start(out=lc_t[:, :, :],
                                            in_=lc[:, :, dlo:dlo + DSLAB])

                nch = npts // CHUNK
                ps = psum_pool.tile([128, 4, 2, CHUNK], F32, tag="ps")
                for c in range(nch):
                    cs = slice(lo - dlo + c * CHUNK, lo - dlo + (c + 1) * CHUNK)
                    for h in range(2):
                        ks = slice(128 * h, 128 * (h + 1))
                        nc.tensor.matmul(out=ps[:, c, h, :],
                                         lhsT=bab_t[:, 0:2, ks],
                                         rhs=lab_t[:, 0:2, cs],
                                         start=(h == 0), stop=False,
                                         perf_mode=DR)
                        nc.tensor.matmul(out=ps[:, c, h, :],
                                         lhsT=bab_t[:, 2:4, ks],
                                         rhs=lab_t[:, 2:4, cs],
                                         start=False, stop=False,
                                         perf_mode=DR)
                        nc.tensor.matmul(out=ps[:, c, h, :],
                                         lhsT=bc_t[:, :, ks],
                                         rhs=lc_t[:, :, cs],
                                         start=False, stop=(h == 1),
                                         perf_mode=DR)
                e_t = eps_pool.tile([128, 4, 2, CHUNK], F16, tag="e")
                nc.scalar.activation(out=e_t[:, 0:nch, :, :],
                                     in_=ps[:, 0:nch, :, :],
                                     func=mybir.ActivationFunctionType.Exp)
                if pending is not None:
                    emit_sums(*pending)
                    pc = pending[2] + pending[3]
                    if pc == 64:
                        nc.scalar.dma_start(out=out[:, 0:64],
                                            in_=s_all[:, 0:64])
                    elif pc == 112:
                        nc.scalar.dma_start(out=out[:, 64:112],
                                            in_=s_all[:, 64:112])
                pending = (ps, e_t, lo // 128, npts // 128)
                lo += npts
            emit_sums(*pending)
            # the final ln(s) - C runs on host; ship the sums
            nc.sync.dma_start(out=out[:, 112:], in_=s_all[:, 112:])
    return nc


def _get_module():
    if "nc" not in _CACHE:
        nc = bacc.Bacc("TRN2", target_bir_lowering=False, debug=False,
                       num_devices=NCORES)
        _build(nc)
        # During the act-table placement pass only, hide every func set except
        # natural_log_exp_and_others (its real act_info index is preserved),
        # so Exp and Ln share one table load instead of reloading before the
        # final Ln.  Restored immediately -- runtime sees the true tables.
        import concourse.hw_specs as _hw_specs
        _orig = _hw_specs.get_activation_tables

        def _patched(arch):
            return {name: (funcs if name == "natural_log_exp_and_others"
                           else set())
                    for name, funcs in _orig(arch).items()}

        if False:  # runtime rejects set 6 (NRT_EXEC_UNIT_UNRECOVERABLE)
            _hw_specs.get_activation_tables = _patched
            bacc.get_activation_tables = _patched
        try:
            nc.compile()
        finally:
            _hw_specs.get_activation_tables = _orig
            bacc.get_activation_tables = _orig
        nc.m = get_hw_module(nc.m)
        _CACHE["nc"] = nc
    return _CACHE["nc"]


def _host_params(centers, covs_inv_sqrt, weights, threshold):
    """Per-cluster parameter rows (B) scaled for fp8, plus exp(-C)."""
    S = covs_inv_sqrt.astype(np.float64)
    w = np.abs(weights.astype(np.float64))
    cp = w / (w.sum() + 1e-30)
    A = np.einsum("kde,kfe->kdf", S, S)
    _, logdetS = np.linalg.slogdet(S)
    logcoef = np.log(np.maximum(cp, 1e-300)) + logdetS
    cen = centers.astype(np.float64)
    m = np.einsum("kde,ke->kd", A, cen)
    t_cAc = np.einsum("kd,kd->k", m, cen)
    thr = float(threshold[0])
    C = 4.0 - (logcoef.max() - thr)
    bias = logcoef - 0.5 * t_cAc - thr + C

    Brows = np.zeros((NROWS, K), np.float64)
    r = 0
    for o in range(1, 16):                      # 480 off-diag product rows
        for i in range(32):
            Brows[r] = -A[:, i, (i + o) % 32] * SCL_P
            r += 1
    for i in range(16):                         # 16 distance-16 pairs
        Brows[r] = -A[:, i, i + 16] * SCL_P
        r += 1
    for i in range(32):                         # squares
        Brows[r] = -0.5 * A[:, i, i] * SCL_D
        r += 1
    for i in range(32):                         # linear
        Brows[r] = m[:, i] * SCL_L
        r += 1
    f8 = ml_dtypes.float8_e4m3
    b0 = bias.astype(f8).astype(np.float64)
    b1 = (bias - b0).astype(f8).astype(np.float64)
    Brows[r] = b0
    Brows[r + 1] = b1
    Brows[r + 2] = bias - b0 - b1
    # row r+3 stays zero (pad)
    return Brows.astype(np.float32), np.float64(C)


def _host_lift(pts):
    """fp8-ready lift planes [NROWS, npts] for one core's points [npts, 32]."""
    X = np.ascontiguousarray(pts.T)             # [32, npts]
    npts = X.shape[1]
    L = np.empty((NROWS, npts), np.float32)
    r = 0
    inv_p = np.float32(1.0 / SCL_P)
    for o in range(1, 16):
        L[r:r + 32] = X * np.roll(X, -o, axis=0) * inv_p
        r += 32
    L[r:r + 16] = X[:16] * X[16:] * inv_p
    r += 16
    L[r:r + 32] = X * X * np.float32(1.0 / SCL_D)
    r += 32
    L[r:r + 32] = X * np.float32(1.0 / SCL_L)
    r += 32
    L[r:r + 2] = 1.0
    L[r + 2] = 1.0
    L[r + 3] = 0.0
    return L


def kernel(points, centers, covs_inv_sqrt, weights, threshold):
    points = np.asarray(points, dtype=np.float32)
    Brows, C = _host_params(np.asarray(centers),
                            np.asarray(covs_inv_sqrt),
                            np.asarray(weights), np.asarray(threshold))
    f8 = ml_dtypes.float8_e4m3
    B8 = Brows.astype(f8)
    bab = np.ascontiguousarray(
        B8[:512].reshape(4, 128, K).transpose(1, 0, 2))
    bc = np.ascontiguousarray(
        B8[512:].reshape(2, CROWS, K).transpose(1, 0, 2))
    in_maps = []
    for r in range(NCORES):
        L8 = _host_lift(points[r * NC_PTS:(r + 1) * NC_PTS]).astype(f8)
        lab = np.ascontiguousarray(
            L8[:512].reshape(4, 128, NC_PTS).transpose(1, 0, 2))
        lc = np.ascontiguousarray(
            L8[512:].reshape(2, CROWS, NC_PTS).transpose(1, 0, 2))
        in_maps.append({"lab": lab, "lc": lc, "bab": bab, "bc": bc})

    nc = _get_module()
    res = bass_utils.run_bass_kernel_spmd(nc, in_maps,
                                          core_ids=list(range(NCORES)))
    sv = np.concatenate([res.results[r]["out"].T.reshape(-1)
                         for r in range(NCORES)])
    ll = np.log(sv.astype(np.float64)) - C
    return ll.reshape(N, 1).astype(np.float32)


# revision 41
# speedup vs baseline: 1.0119x; 1.0018x over previous
"""Gaussian-mixture log-likelihood kernel for 8 Trainium2 NeuronCores.

Math: ll_i = logsumexp_j( -0.5 x_i^T A_j x_i + x_i^T m_j + bias_j ) - C
with A_j = S_j S_j^T.  The quadratic form is a single contraction of 564
"lift" rows per point against a [564, K] parameter matrix: 496 unique
symmetric-pair products, 32 squares, 32 linear rows, 3 bias rows (split so
fp8 quantization of the bias is exact to ~1e-3) and 1 zero pad.  Lift rows
and parameters are stored in fp8-e4m3 with per-row-type power-of-two scales
folded oppositely into the two factors, so the PE runs DoubleRow matmuls
(256-row contraction per instruction, 2x column rate).

Layout is K-on-partitions: the parameter matrix is the stationary operand
and scores land as [K-half, points] in PSUM, 6 matmuls per 256-point chunk.
One [128, 2048] Exp turns a 4-chunk PSUM slab into f16 likelihoods; the sum
over K is then a second matmul pass (e-tile stationary x a ones column,
output free size 1), which costs the PE almost nothing and keeps the sum in
f32.  A global shift C (folded into the bias rows) makes exp() safe without
a per-point max; one Ln + bias-add finishes all 16384 points per core.

The lift planes are packed host-side (cheap: O(N D^2) vs the device's
O(N K D^2) contraction) and shipped as fp8, which also removes all
shuffle/multiply traffic from DVE/Pool.

Sharding: data-parallel over points, 16384 points/core; K-sized parameters
are replicated (precomputed on host in float64 -- tiny vs the N*K work).
"""

import sys

sys.path.insert(0, "/opt/trn_rl_repo")

import numpy as np
import ml_dtypes

import concourse.bass as bass
import concourse.bacc as bacc
import concourse.mybir as mybir
from concourse import bass_utils
from concourse.bass_interp import get_hw_module
from concourse.tile import TileContext

N, K, D = 131072, 256, 32
NCORES = 8
NC_PTS = N // NCORES            # 16384 points per core
NTILES = NC_PTS // 128          # 128 output columns
DSLAB = 1024                    # points per DMA slab
NDMA = NC_PTS // DSLAB          # 16
PSLAB = 1024                    # points per PSUM slab (4 chunks x 2 K-halves)
CHUNK = 256                     # points per matmul chunk
F32 = mybir.dt.float32
F16 = mybir.dt.float16
F8 = mybir.dt.float8e4

NROWS = 564                     # 512 (chunks A,B) + 52 (chunk C)
CROWS = 26                      # chunk C pair rows

SCL_P = 16.0                    # off-diagonal product rows
SCL_D = 4.0                     # diagonal (square) rows
SCL_L = 4.0                     # linear rows

_CACHE = {}


def _build(nc):
    lab = nc.dram_tensor("lab", [128, 4, NC_PTS], F8, kind="ExternalInput").ap()
    lc = nc.dram_tensor("lc", [CROWS, 2, NC_PTS], F8, kind="ExternalInput").ap()
    bab = nc.dram_tensor("bab", [128, 4, K], F8, kind="ExternalInput").ap()
    bc = nc.dram_tensor("bc", [CROWS, 2, K], F8, kind="ExternalInput").ap()
    out = nc.dram_tensor("out", [128, NTILES], F32, kind="ExternalOutput").ap()

    DR = mybir.MatmulPerfMode.DoubleRow

    with TileContext(nc) as tc:
        with (
            tc.tile_pool(name="par", bufs=1) as par_pool,
            tc.tile_pool(name="src", bufs=4) as src_pool,
            tc.tile_pool(name="eps", bufs=6) as eps_pool,
            tc.tile_pool(name="acc", bufs=1) as acc_pool,
            tc.tile_pool(name="psum", bufs=2, space="PSUM") as psum_pool,
        ):
            # split param issue across the SP and ACT DGE queues so the lift
            # stream and the (tiny) params overlap at startup.
            bab_t = par_pool.tile([128, 4, K], F8, tag="bab")
            bc_t = par_pool.tile([CROWS, 2, K], F8, tag="bc")
            ones = par_pool.tile([128, 1], F16, tag="ones")
            warm = par_pool.tile([128, 512], F16, tag="warm")
            nc.sync.dma_start(out=bab_t[:, :, :], in_=bab[:, :, :])
            nc.scalar.dma_start(out=bc_t[:, :, :], in_=bc[:, :, :])
            nc.gpsimd.memset(ones[:, :], 1.0)
            nc.gpsimd.memset(warm[:, :], 0.0)

            s_all = acc_pool.tile([128, NTILES], F32, tag="s_all")

            # ramp the PE p-state on junk during the initial DMA window so
            # the first real matmuls run at (nearly) full clock
            ps_warm = psum_pool.tile([128, 4, 2, CHUNK], F32, tag="ps")
            for _ in range(3):
                nc.tensor.matmul(out=ps_warm[0:1, 0, :, :],
                                 lhsT=warm[:, 0:1], rhs=warm[:, :],
                                 start=True, stop=True)

            def emit_sums(ps, e_t, col, ntiles):
                # sum over K: e-tile stationary x ones column -> [128, 1] per
                # point-tile, accumulated into columns of bank 0 of the (now
                # dead) score psum.  Emitted one slab late so the in-order PE
                # stream never stalls on the Exp of its own slab.
                for u in range(ntiles):
                    c, w = u // 2, u % 2
                    sl = slice(128 * w, 128 * (w + 1))
                    for h in range(2):
                        nc.tensor.matmul(out=ps[:, 0, 0, u:u + 1],
                                         lhsT=e_t[:, c, h, sl],
                                         rhs=ones[:, :],
                                         start=(u == 0 and h == 0),
                                         stop=(u == ntiles - 1 and h == 1))
                nc.vector.tensor_copy(out=s_all[:, col:col + ntiles],
                                      in_=ps[:, 0, 0, 0:ntiles])

            # 512-point slabs at both ends shorten the pipeline fill (first
            # Exp waits on half the data) and drain; 1024-point slabs in the
            # middle amortize the ACT per-instruction overhead.
            sizes = [1024] * 16
            pending = None
            lo = 0
            for npts in sizes:
                dlo = lo - lo % DSLAB
                if lo == dlo:  # first slab touching this DMA tile loads it
                    lab_t = src_pool.tile([128, 4, DSLAB], F8, tag="lab")
                    lc_t = src_pool.tile([CROWS, 2, DSLAB], F8, tag="lc")
                    if lo == 0:
                        nc.sync.dma_start(out=lab_t[:, :, 0:512],
                                          in_=lab[:, :, 0:512])
                        nc.gpsimd.dma_start(out=lc_t[:, :, :],
                                            in_=lc[:, :, 0:DSLAB])
                        nc.sync.dma_start(out=lab_t[:, :, 512:DSLAB],
                                          in_=lab[:, :, 512:DSLAB])
                    else:
                        nc.sync.dma_start(out=lab_t[:, :, :],
                                          in_=lab[:, :, dlo:dlo + DSLAB])
                        nc.gpsimd.dma_start(out=lc_t[:, :, :],
                                            in_=lc[:, :, dlo:dlo + DSLAB])

                nch = npts // CHUNK
                ps = psum_pool.tile([128, 4, 2, CHUNK], F32, tag="ps")
                for c in range(nch):
                    cs = slice(lo - dlo + c * CHUNK, lo - dlo + (c + 1) * CHUNK)
                    for h in range(2):
                        ks = slice(128 * h, 128 * (h + 1))
                        nc.tensor.matmul(out=ps[:, c, h, :],
                                         lhsT=bab_t[:, 0:2, ks],
                                         rhs=lab_t[:, 0:2, cs],
                                         start=(h == 0), stop=False,
                                         perf_mode=DR)
                        nc.tensor.matmul(out=ps[:, c, h, :],
                                         lhsT=bab_t[:, 2:4, ks],
                                         rhs=lab_t[:, 2:4, cs],
                                         start=False, stop=False,
                                         perf_mode=DR)
                        nc.tensor.matmul(out=ps[:, c, h, :],
                                         lhsT=bc_t[:, :, ks],
                                         rhs=lc_t[:, :, cs],
                                         start=False, stop=(h == 1),
                                         perf_mode=DR)
                e_t = eps_pool.tile([128, 4, 2, CHUNK], F16, tag="e")
                nc.scalar.activation(out=e_t[:, 0:nch, :, :],
                                     in_=ps[:, 0:nch, :, :],
                                     func=mybir.ActivationFunctionType.Exp)
                if pending is not None:
                    emit_sums(*pending)
                    pc = pending[2] + pending[3]
                    if pc == 64:
                        nc.scalar.dma_start(out=out[:, 0:64],
                                            in_=s_all[:, 0:64])
                    elif pc == 112:
                        nc.scalar.dma_start(out=out[:, 64:112],
                                            in_=s_all[:, 64:112])
                pending = (ps, e_t, lo // 128, npts // 128)
                lo += npts
            emit_sums(*pending)
            # the final ln(s) - C runs on host; ship the sums
            nc.sync.dma_start(out=out[:, 112:], in_=s_all[:, 112:])
    return nc


def _get_module():
    if "nc" not in _CACHE:
        nc = bacc.Bacc("TRN2", target_bir_lowering=False, debug=False,
                       num_devices=NCORES)
        _build(nc)
        # During the act-table placement pass only, hide every func set except
        # natural_log_exp_and_others (its real act_info index is preserved),
        # so Exp and Ln share one table load instead of reloading before the
        # final Ln.  Restored immediately -- runtime sees the true tables.
        import concourse.hw_specs as _hw_specs
        _orig = _hw_specs.get_activation_tables

        def _patched(arch):
            return {name: (funcs if name == "natural_log_exp_and_others"
                           else set())
                    for name, funcs in _orig(arch).items()}

        if False:  # runtime rejects set 6 (NRT_EXEC_UNIT_UNRECOVERABLE)
            _hw_specs.get_activation_tables = _patched
            bacc.get_activation_tables = _patched
        try:
            nc.compile()
        finally:
            _hw_specs.get_activation_tables = _orig
            bacc.get_activation_tables = _orig
        nc.m = get_hw_module(nc.m)
        _CACHE["nc"] = nc
    return _CACHE["nc"]


def _host_params(centers, covs_inv_sqrt, weights, threshold):
    """Per-cluster parameter rows (B) scaled for fp8, plus exp(-C)."""
    S = covs_inv_sqrt.astype(np.float64)
    w = np.abs(weights.astype(np.float64))
    cp = w / (w.sum() + 1e-30)
    A = np.einsum("kde,kfe->kdf", S, S)
    _, logdetS = np.linalg.slogdet(S)
    logcoef = np.log(np.maximum(cp, 1e-300)) + logdetS
    cen = centers.astype(np.float64)
    m = np.einsum("kde,ke->kd", A, cen)
    t_cAc = np.einsum("kd,kd->k", m, cen)
    thr = float(threshold[0])
    C = 4.0 - (logcoef.max() - thr)
    bias = logcoef - 0.5 * t_cAc - thr + C

    Brows = np.zeros((NROWS, K), np.float64)
    r = 0
    for o in range(1, 16):                      # 480 off-diag product rows
        for i in range(32):
            Brows[r] = -A[:, i, (i + o) % 32] * SCL_P
            r += 1
    for i in range(16):                         # 16 distance-16 pairs
        Brows[r] = -A[:, i, i + 16] * SCL_P
        r += 1
    for i in range(32):                         # squares
        Brows[r] = -0.5 * A[:, i, i] * SCL_D
        r += 1
    for i in range(32):                         # linear
        Brows[r] = m[:, i] * SCL_L
        r += 1
    f8 = ml_dtypes.float8_e4m3
    b0 = bias.astype(f8).astype(np.float64)
    b1 = (bias - b0).astype(f8).astype(np.float64)
    Brows[r] = b0
    Brows[r + 1] = b1
    Brows[r + 2] = bias - b0 - b1
    # row r+3 stays zero (pad)
    return Brows.astype(np.float32), np.float64(C)


def _host_lift(pts):
    """fp8-ready lift planes [NROWS, npts] for one core's points [npts, 32]."""
    X = np.ascontiguousarray(pts.T)             # [32, npts]
    npts = X.shape[1]
    L = np.empty((NROWS, npts), np.float32)
    r = 0
    inv_p = np.float32(1.0 / SCL_P)
    for o in range(1, 16):
        L[r:r + 32] = X * np.roll(X, -o, axis=0) * inv_p
        r += 32
    L[r:r + 16] = X[:16] * X[16:] * inv_p
    r += 16
    L[r:r + 32] = X * X * np.float32(1.0 / SCL_D)
    r += 32
    L[r:r + 32] = X * np.float32(1.0 / SCL_L)
    r += 32
    L[r:r + 2] = 1.0
    L[r + 2] = 1.0
    L[r + 3] = 0.0
    return L


def kernel(points, centers, covs_inv_sqrt, weights, threshold):
    points = np.asarray(points, dtype=np.float32)
    Brows, C = _host_params(np.asarray(centers),
                            np.asarray(covs_inv_sqrt),
                            np.asarray(weights), np.asarray(threshold))
    f8 = ml_dtypes.float8_e4m3
    B8 = Brows.astype(f8)
    bab = np.ascontiguousarray(
        B8[:512].reshape(4, 128, K).transpose(1, 0, 2))
    bc = np.ascontiguousarray(
        B8[512:].reshape(2, CROWS, K).transpose(1, 0, 2))
    in_maps = []
    for r in range(NCORES):
        L8 = _host_lift(points[r * NC_PTS:(r + 1) * NC_PTS]).astype(f8)
        lab = np.ascontiguousarray(
            L8[:512].reshape(4, 128, NC_PTS).transpose(1, 0, 2))
        lc = np.ascontiguousarray(
            L8[512:].reshape(2, CROWS, NC_PTS).transpose(1, 0, 2))
        in_maps.append({"lab": lab, "lc": lc, "bab": bab, "bc": bc})

    nc = _get_module()
    res = bass_utils.run_bass_kernel_spmd(nc, in_maps,
                                          core_ids=list(range(NCORES)))
    sv = np.concatenate([res.results[r]["out"].T.reshape(-1)
                         for r in range(NCORES)])
    ll = np.log(sv.astype(np.float64)) - C
    return ll.reshape(N, 1).astype(np.float32)


# revision 42
# speedup vs baseline: 1.0137x; 1.0018x over previous
"""Gaussian-mixture log-likelihood kernel for 8 Trainium2 NeuronCores.

Math: ll_i = logsumexp_j( -0.5 x_i^T A_j x_i + x_i^T m_j + bias_j ) - C
with A_j = S_j S_j^T.  The quadratic form is a single contraction of 564
"lift" rows per point against a [564, K] parameter matrix: 496 unique
symmetric-pair products, 32 squares, 32 linear rows, 3 bias rows (split so
fp8 quantization of the bias is exact to ~1e-3) and 1 zero pad.  Lift rows
and parameters are stored in fp8-e4m3 with per-row-type power-of-two scales
folded oppositely into the two factors, so the PE runs DoubleRow matmuls
(256-row contraction per instruction, 2x column rate).

Layout is K-on-partitions: the parameter matrix is the stationary operand
and scores land as [K-half, points] in PSUM, 6 matmuls per 256-point chunk.
One [128, 2048] Exp turns a 4-chunk PSUM slab into f16 likelihoods; the sum
over K is then a second matmul pass (e-tile stationary x a ones column,
output free size 1), which costs the PE almost nothing and keeps the sum in
f32.  A global shift C (folded into the bias rows) makes exp() safe without
a per-point max; one Ln + bias-add finishes all 16384 points per core.

The lift planes are packed host-side (cheap: O(N D^2) vs the device's
O(N K D^2) contraction) and shipped as fp8, which also removes all
shuffle/multiply traffic from DVE/Pool.

Sharding: data-parallel over points, 16384 points/core; K-sized parameters
are replicated (precomputed on host in float64 -- tiny vs the N*K work).
"""

import sys

sys.path.insert(0, "/opt/trn_rl_repo")

import numpy as np
import ml_dtypes

import concourse.bass as bass
import concourse.bacc as bacc
import concourse.mybir as mybir
from concourse import bass_utils
from concourse.bass_interp import get_hw_module
from concourse.tile import TileContext

N, K, D = 131072, 256, 32
NCORES = 8
NC_PTS = N // NCORES            # 16384 points per core
NTILES = NC_PTS // 128          # 128 output columns
DSLAB = 1024                    # points per DMA slab
NDMA = NC_PTS // DSLAB          # 16
PSLAB = 1024                    # points per PSUM slab (4 chunks x 2 K-halves)
CHUNK = 256                     # points per matmul chunk
F32 = mybir.dt.float32
F16 = mybir.dt.float16
F8 = mybir.dt.float8e4

NROWS = 564                     # 512 (chunks A,B) + 52 (chunk C)
CROWS = 26                      # chunk C pair rows

SCL_P = 16.0                    # off-diagonal product rows
SCL_D = 4.0                     # diagonal (square) rows
SCL_L = 4.0                     # linear rows

_CACHE = {}


def _build(nc):
    lab = nc.dram_tensor("lab", [128, 4, NC_PTS], F8, kind="ExternalInput").ap()
    lc = nc.dram_tensor("lc", [CROWS, 2, NC_PTS], F8, kind="ExternalInput").ap()
    bab = nc.dram_tensor("bab", [128, 4, K], F8, kind="ExternalInput").ap()
    bc = nc.dram_tensor("bc", [CROWS, 2, K], F8, kind="ExternalInput").ap()
    out = nc.dram_tensor("out", [128, NTILES], F32, kind="ExternalOutput").ap()

    DR = mybir.MatmulPerfMode.DoubleRow

    with TileContext(nc) as tc:
        with (
            tc.tile_pool(name="par", bufs=1) as par_pool,
            tc.tile_pool(name="src", bufs=6) as src_pool,
            tc.tile_pool(name="eps", bufs=8) as eps_pool,
            tc.tile_pool(name="acc", bufs=1) as acc_pool,
            tc.tile_pool(name="psum", bufs=2, space="PSUM") as psum_pool,
        ):
            # split param issue across the SP and ACT DGE queues so the lift
            # stream and the (tiny) params overlap at startup.
            bab_t = par_pool.tile([128, 4, K], F8, tag="bab")
            bc_t = par_pool.tile([CROWS, 2, K], F8, tag="bc")
            ones = par_pool.tile([128, 1], F16, tag="ones")
            warm = par_pool.tile([128, 512], F16, tag="warm")
            nc.sync.dma_start(out=bab_t[:, :, :], in_=bab[:, :, :])
            nc.scalar.dma_start(out=bc_t[:, :, :], in_=bc[:, :, :])
            nc.gpsimd.memset(ones[:, :], 1.0)
            nc.gpsimd.memset(warm[:, :], 0.0)

            s_all = acc_pool.tile([128, NTILES], F32, tag="s_all")

            # ramp the PE p-state on junk during the initial DMA window so
            # the first real matmuls run at (nearly) full clock
            ps_warm = psum_pool.tile([128, 4, 2, CHUNK], F32, tag="ps")
            for _ in range(3):
                nc.tensor.matmul(out=ps_warm[0:1, 0, :, :],
                                 lhsT=warm[:, 0:1], rhs=warm[:, :],
                                 start=True, stop=True)

            def emit_sums(ps, e_t, col, ntiles):
                # sum over K: e-tile stationary x ones column -> [128, 1] per
                # point-tile, accumulated into columns of bank 0 of the (now
                # dead) score psum.  Emitted one slab late so the in-order PE
                # stream never stalls on the Exp of its own slab.
                for u in range(ntiles):
                    c, w = u // 2, u % 2
                    sl = slice(128 * w, 128 * (w + 1))
                    for h in range(2):
                        nc.tensor.matmul(out=ps[:, 0, 0, u:u + 1],
                                         lhsT=e_t[:, c, h, sl],
                                         rhs=ones[:, :],
                                         start=(u == 0 and h == 0),
                                         stop=(u == ntiles - 1 and h == 1))
                nc.vector.tensor_copy(out=s_all[:, col:col + ntiles],
                                      in_=ps[:, 0, 0, 0:ntiles])

            # 512-point slabs at both ends shorten the pipeline fill (first
            # Exp waits on half the data) and drain; 1024-point slabs in the
            # middle amortize the ACT per-instruction overhead.
            sizes = [1024] * 16
            pending = None
            lo = 0
            for npts in sizes:
                dlo = lo - lo % DSLAB
                if lo == dlo:  # first slab touching this DMA tile loads it
                    lab_t = src_pool.tile([128, 4, DSLAB], F8, tag="lab")
                    lc_t = src_pool.tile([CROWS, 2, DSLAB], F8, tag="lc")
                    if lo == 0:
                        nc.sync.dma_start(out=lab_t[:, :, 0:512],
                                          in_=lab[:, :, 0:512])
                        nc.gpsimd.dma_start(out=lc_t[:, :, :],
                                            in_=lc[:, :, 0:DSLAB])
                        nc.sync.dma_start(out=lab_t[:, :, 512:DSLAB],
                                          in_=lab[:, :, 512:DSLAB])
                    else:
                        nc.sync.dma_start(out=lab_t[:, :, :],
                                          in_=lab[:, :, dlo:dlo + DSLAB])
                        nc.gpsimd.dma_start(out=lc_t[:, :, :],
                                            in_=lc[:, :, dlo:dlo + DSLAB])

                nch = npts // CHUNK
                ps = psum_pool.tile([128, 4, 2, CHUNK], F32, tag="ps")
                for c in range(nch):
                    cs = slice(lo - dlo + c * CHUNK, lo - dlo + (c + 1) * CHUNK)
                    for h in range(2):
                        ks = slice(128 * h, 128 * (h + 1))
                        nc.tensor.matmul(out=ps[:, c, h, :],
                                         lhsT=bab_t[:, 0:2, ks],
                                         rhs=lab_t[:, 0:2, cs],
                                         start=(h == 0), stop=False,
                                         perf_mode=DR)
                        nc.tensor.matmul(out=ps[:, c, h, :],
                                         lhsT=bab_t[:, 2:4, ks],
                                         rhs=lab_t[:, 2:4, cs],
                                         start=False, stop=False,
                                         perf_mode=DR)
                        nc.tensor.matmul(out=ps[:, c, h, :],
                                         lhsT=bc_t[:, :, ks],
                                         rhs=lc_t[:, :, cs],
                                         start=False, stop=(h == 1),
                                         perf_mode=DR)
                e_t = eps_pool.tile([128, 4, 2, CHUNK], F16, tag="e")
                nc.scalar.activation(out=e_t[:, 0:nch, :, :],
                                     in_=ps[:, 0:nch, :, :],
                                     func=mybir.ActivationFunctionType.Exp)
                if pending is not None:
                    emit_sums(*pending)
                    pc = pending[2] + pending[3]
                    if pc == 64:
                        nc.scalar.dma_start(out=out[:, 0:64],
                                            in_=s_all[:, 0:64])
                    elif pc == 112:
                        nc.scalar.dma_start(out=out[:, 64:112],
                                            in_=s_all[:, 64:112])
                pending = (ps, e_t, lo // 128, npts // 128)
                lo += npts
            emit_sums(*pending)
            # the final ln(s) - C runs on host; ship the sums
            nc.sync.dma_start(out=out[:, 112:], in_=s_all[:, 112:])
    return nc


def _get_module():
    if "nc" not in _CACHE:
        nc = bacc.Bacc("TRN2", target_bir_lowering=False, debug=False,
                       num_devices=NCORES)
        _build(nc)
        # During the act-table placement pass only, hide every func set except
        # natural_log_exp_and_others (its real act_info index is preserved),
        # so Exp and Ln share one table load instead of reloading before the
        # final Ln.  Restored immediately -- runtime sees the true tables.
        import concourse.hw_specs as _hw_specs
        _orig = _hw_specs.get_activation_tables

        def _patched(arch):
            return {name: (funcs if name == "natural_log_exp_and_others"
                           else set())
                    for name, funcs in _orig(arch).items()}

        if False:  # runtime rejects set 6 (NRT_EXEC_UNIT_UNRECOVERABLE)
            _hw_specs.get_activation_tables = _patched
            bacc.get_activation_tables = _patched
        try:
            nc.compile()
        finally:
            _hw_specs.get_activation_tables = _orig
            bacc.get_activation_tables = _orig
        nc.m = get_hw_module(nc.m)
        _CACHE["nc"] = nc
    return _CACHE["nc"]


def _host_params(centers, covs_inv_sqrt, weights, threshold):
    """Per-cluster parameter rows (B) scaled for fp8, plus exp(-C)."""
    S = covs_inv_sqrt.astype(np.float64)
    w = np.abs(weights.astype(np.float64))
    cp = w / (w.sum() + 1e-30)
    A = np.einsum("kde,kfe->kdf", S, S)
    _, logdetS = np.linalg.slogdet(S)
    logcoef = np.log(np.maximum(cp, 1e-300)) + logdetS
    cen = centers.astype(np.float64)
    m = np.einsum("kde,ke->kd", A, cen)
    t_cAc = np.einsum("kd,kd->k", m, cen)
    thr = float(threshold[0])
    C = 4.0 - (logcoef.max() - thr)
    bias = logcoef - 0.5 * t_cAc - thr + C

    Brows = np.zeros((NROWS, K), np.float64)
    r = 0
    for o in range(1, 16):                      # 480 off-diag product rows
        for i in range(32):
            Brows[r] = -A[:, i, (i + o) % 32] * SCL_P
            r += 1
    for i in range(16):                         # 16 distance-16 pairs
        Brows[r] = -A[:, i, i + 16] * SCL_P
        r += 1
    for i in range(32):                         # squares
        Brows[r] = -0.5 * A[:, i, i] * SCL_D
        r += 1
    for i in range(32):                         # linear
        Brows[r] = m[:, i] * SCL_L
        r += 1
    f8 = ml_dtypes.float8_e4m3
    b0 = bias.astype(f8).astype(np.float64)
    b1 = (bias - b0).astype(f8).astype(np.float64)
    Brows[r] = b0
    Brows[r + 1] = b1
    Brows[r + 2] = bias - b0 - b1
    # row r+3 stays zero (pad)
    return Brows.astype(np.float32), np.float64(C)


def _host_lift(pts):
    """fp8-ready lift planes [NROWS, npts] for one core's points [npts, 32]."""
    X = np.ascontiguousarray(pts.T)             # [32, npts]
    npts = X.shape[1]
    L = np.empty((NROWS, npts), np.float32)
    r = 0
    inv_p = np.float32(1.0 / SCL_P)
    for o in range(1, 16):
        L[r:r + 32] = X * np.roll(X, -o, axis=0) * inv_p
        r += 32
    L[r:r + 16] = X[:16] * X[16:] * inv_p
    r += 16
    L[r:r + 32] = X * X * np.float32(1.0 / SCL_D)
    r += 32
    L[r:r + 32] = X * np.float32(1.0 / SCL_L)
    r += 32
    L[r:r + 2] = 1.0
    L[r + 2] = 1.0
    L[r + 3] = 0.0
    return L


def kernel(points, centers, covs_inv_sqrt, weights, threshold):
    points = np.asarray(points, dtype=np.float32)
    Brows, C = _host_params(np.asarray(centers),
                            np.asarray(covs_inv_sqrt),
                            np.asarray(weights), np.asarray(threshold))
    f8 = ml_dtypes.float8_e4m3
    B8 = Brows.astype(f8)
    bab = np.ascontiguousarray(
        B8[:512].reshape(4, 128, K).transpose(1, 0, 2))
    bc = np.ascontiguousarray(
        B8[512:].reshape(2, CROWS, K).transpose(1, 0, 2))
    in_maps = []
    for r in range(NCORES):
        L8 = _host_lift(points[r * NC_PTS:(r + 1) * NC_PTS]).astype(f8)
        lab = np.ascontiguousarray(
            L8[:512].reshape(4, 128, NC_PTS).transpose(1, 0, 2))
        lc = np.ascontiguousarray(
            L8[512:].reshape(2, CROWS, NC_PTS).transpose(1, 0, 2))
        in_maps.append({"lab": lab, "lc": lc, "bab": bab, "bc": bc})

    nc = _get_module()
    res = bass_utils.run_bass_kernel_spmd(nc, in_maps,
                                          core_ids=list(range(NCORES)))
    sv = np.concatenate([res.results[r]["out"].T.reshape(-1)
                         for r in range(NCORES)])
    ll = np.log(sv.astype(np.float64)) - C
    return ll.reshape(N, 1).astype(np.float32)


# revision 43
# speedup vs baseline: 1.0173x; 1.0035x over previous
"""Gaussian-mixture log-likelihood kernel for 8 Trainium2 NeuronCores.

Math: ll_i = logsumexp_j( -0.5 x_i^T A_j x_i + x_i^T m_j + bias_j ) - C
with A_j = S_j S_j^T.  The quadratic form is a single contraction of 564
"lift" rows per point against a [564, K] parameter matrix: 496 unique
symmetric-pair products, 32 squares, 32 linear rows, 3 bias rows (split so
fp8 quantization of the bias is exact to ~1e-3) and 1 zero pad.  Lift rows
and parameters are stored in fp8-e4m3 with per-row-type power-of-two scales
folded oppositely into the two factors, so the PE runs DoubleRow matmuls
(256-row contraction per instruction, 2x column rate).

Layout is K-on-partitions: the parameter matrix is the stationary operand
and scores land as [K-half, points] in PSUM, 6 matmuls per 256-point chunk.
One [128, 2048] Exp turns a 4-chunk PSUM slab into f16 likelihoods; the sum
over K is then a second matmul pass (e-tile stationary x a ones column,
output free size 1), which costs the PE almost nothing and keeps the sum in
f32.  A global shift C (folded into the bias rows) makes exp() safe without
a per-point max; one Ln + bias-add finishes all 16384 points per core.

The lift planes are packed host-side (cheap: O(N D^2) vs the device's
O(N K D^2) contraction) and shipped as fp8, which also removes all
shuffle/multiply traffic from DVE/Pool.

Sharding: data-parallel over points, 16384 points/core; K-sized parameters
are replicated (precomputed on host in float64 -- tiny vs the N*K work).
"""

import sys

sys.path.insert(0, "/opt/trn_rl_repo")

import numpy as np
import ml_dtypes

import concourse.bass as bass
import concourse.bacc as bacc
import concourse.mybir as mybir
from concourse import bass_utils
from concourse.bass_interp import get_hw_module
from concourse.tile import TileContext

N, K, D = 131072, 256, 32
NCORES = 8
NC_PTS = N // NCORES            # 16384 points per core
NTILES = NC_PTS // 128          # 128 output columns
DSLAB = 1024                    # points per DMA slab
NDMA = NC_PTS // DSLAB          # 16
PSLAB = 1024                    # points per PSUM slab (4 chunks x 2 K-halves)
CHUNK = 256                     # points per matmul chunk
F32 = mybir.dt.float32
F16 = mybir.dt.float16
F8 = mybir.dt.float8e4

NROWS = 564                     # 512 (chunks A,B) + 52 (chunk C)
CROWS = 26                      # chunk C pair rows

SCL_P = 16.0                    # off-diagonal product rows
SCL_D = 4.0                     # diagonal (square) rows
SCL_L = 4.0                     # linear rows

_CACHE = {}


def _build(nc):
    lab = nc.dram_tensor("lab", [128, 4, NC_PTS], F8, kind="ExternalInput").ap()
    lc = nc.dram_tensor("lc", [CROWS, 2, NC_PTS], F8, kind="ExternalInput").ap()
    bab = nc.dram_tensor("bab", [128, 4, K], F8, kind="ExternalInput").ap()
    bc = nc.dram_tensor("bc", [CROWS, 2, K], F8, kind="ExternalInput").ap()
    out = nc.dram_tensor("out", [128, NTILES], F32, kind="ExternalOutput").ap()

    DR = mybir.MatmulPerfMode.DoubleRow

    with TileContext(nc) as tc:
        with (
            tc.tile_pool(name="par", bufs=1) as par_pool,
            tc.tile_pool(name="src", bufs=10) as src_pool,
            tc.tile_pool(name="eps", bufs=12) as eps_pool,
            tc.tile_pool(name="acc", bufs=1) as acc_pool,
            tc.tile_pool(name="psum", bufs=2, space="PSUM") as psum_pool,
        ):
            # split param issue across the SP and ACT DGE queues so the lift
            # stream and the (tiny) params overlap at startup.
            bab_t = par_pool.tile([128, 4, K], F8, tag="bab")
            bc_t = par_pool.tile([CROWS, 2, K], F8, tag="bc")
            ones = par_pool.tile([128, 1], F16, tag="ones")
            warm = par_pool.tile([128, 512], F16, tag="warm")
            nc.sync.dma_start(out=bab_t[:, :, :], in_=bab[:, :, :])
            nc.scalar.dma_start(out=bc_t[:, :, :], in_=bc[:, :, :])
            nc.gpsimd.memset(ones[:, :], 1.0)
            nc.gpsimd.memset(warm[:, :], 0.0)

            s_all = acc_pool.tile([128, NTILES], F32, tag="s_all")

            # ramp the PE p-state on junk during the initial DMA window so
            # the first real matmuls run at (nearly) full clock
            ps_warm = psum_pool.tile([128, 4, 2, CHUNK], F32, tag="ps")
            for _ in range(3):
                nc.tensor.matmul(out=ps_warm[0:1, 0, :, :],
                                 lhsT=warm[:, 0:1], rhs=warm[:, :],
                                 start=True, stop=True)

            def emit_sums(ps, e_t, col, ntiles):
                # sum over K: e-tile stationary x ones column -> [128, 1] per
                # point-tile, accumulated into columns of bank 0 of the (now
                # dead) score psum.  Emitted one slab late so the in-order PE
                # stream never stalls on the Exp of its own slab.
                for u in range(ntiles):
                    c, w = u // 2, u % 2
                    sl = slice(128 * w, 128 * (w + 1))
                    for h in range(2):
                        nc.tensor.matmul(out=ps[:, 0, 0, u:u + 1],
                                         lhsT=e_t[:, c, h, sl],
                                         rhs=ones[:, :],
                                         start=(u == 0 and h == 0),
                                         stop=(u == ntiles - 1 and h == 1))
                nc.vector.tensor_copy(out=s_all[:, col:col + ntiles],
                                      in_=ps[:, 0, 0, 0:ntiles])

            # 512-point slabs at both ends shorten the pipeline fill (first
            # Exp waits on half the data) and drain; 1024-point slabs in the
            # middle amortize the ACT per-instruction overhead.
            sizes = [1024] * 16
            pending = None
            lo = 0
            for npts in sizes:
                dlo = lo - lo % DSLAB
                if lo == dlo:  # first slab touching this DMA tile loads it
                    lab_t = src_pool.tile([128, 4, DSLAB], F8, tag="lab")
                    lc_t = src_pool.tile([CROWS, 2, DSLAB], F8, tag="lc")
                    if lo == 0:
                        nc.sync.dma_start(out=lab_t[:, :, 0:512],
                                          in_=lab[:, :, 0:512])
                        nc.gpsimd.dma_start(out=lc_t[:, :, :],
                                            in_=lc[:, :, 0:DSLAB])
                        nc.sync.dma_start(out=lab_t[:, :, 512:DSLAB],
                                          in_=lab[:, :, 512:DSLAB])
                    else:
                        nc.sync.dma_start(out=lab_t[:, :, :],
                                          in_=lab[:, :, dlo:dlo + DSLAB])
                        nc.gpsimd.dma_start(out=lc_t[:, :, :],
                                            in_=lc[:, :, dlo:dlo + DSLAB])

                nch = npts // CHUNK
                ps = psum_pool.tile([128, 4, 2, CHUNK], F32, tag="ps")
                for c in range(nch):
                    cs = slice(lo - dlo + c * CHUNK, lo - dlo + (c + 1) * CHUNK)
                    for h in range(2):
                        ks = slice(128 * h, 128 * (h + 1))
                        nc.tensor.matmul(out=ps[:, c, h, :],
                                         lhsT=bab_t[:, 0:2, ks],
                                         rhs=lab_t[:, 0:2, cs],
                                         start=(h == 0), stop=False,
                                         perf_mode=DR)
                        nc.tensor.matmul(out=ps[:, c, h, :],
                                         lhsT=bab_t[:, 2:4, ks],
                                         rhs=lab_t[:, 2:4, cs],
                                         start=False, stop=False,
                                         perf_mode=DR)
                        nc.tensor.matmul(out=ps[:, c, h, :],
                                         lhsT=bc_t[:, :, ks],
                                         rhs=lc_t[:, :, cs],
                                         start=False, stop=(h == 1),
                                         perf_mode=DR)
                e_t = eps_pool.tile([128, 4, 2, CHUNK], F16, tag="e")
                nc.scalar.activation(out=e_t[:, 0:nch, :, :],
                                     in_=ps[:, 0:nch, :, :],
                                     func=mybir.ActivationFunctionType.Exp)
                if pending is not None:
                    emit_sums(*pending)
                    pc = pending[2] + pending[3]
                    if pc == 64:
                        nc.scalar.dma_start(out=out[:, 0:64],
                                            in_=s_all[:, 0:64])
                    elif pc == 112:
                        nc.scalar.dma_start(out=out[:, 64:112],
                                            in_=s_all[:, 64:112])
                pending = (ps, e_t, lo // 128, npts // 128)
                lo += npts
            emit_sums(*pending)
            # the final ln(s) - C runs on host; ship the sums
            nc.sync.dma_start(out=out[:, 112:], in_=s_all[:, 112:])
    return nc


def _get_module():
    if "nc" not in _CACHE:
        nc = bacc.Bacc("TRN2", target_bir_lowering=False, debug=False,
                       num_devices=NCORES)
        _build(nc)
        # During the act-table placement pass only, hide every func set except
        # natural_log_exp_and_others (its real act_info index is preserved),
        # so Exp and Ln share one table load instead of reloading before the
        # final Ln.  Restored immediately -- runtime sees the true tables.
        import concourse.hw_specs as _hw_specs
        _orig = _hw_specs.get_activation_tables

        def _patched(arch):
            return {name: (funcs if name == "natural_log_exp_and_others"
                           else set())
                    for name, funcs in _orig(arch).items()}

        if False:  # runtime rejects set 6 (NRT_EXEC_UNIT_UNRECOVERABLE)
            _hw_specs.get_activation_tables = _patched
            bacc.get_activation_tables = _patched
        try:
            nc.compile()
        finally:
            _hw_specs.get_activation_tables = _orig
            bacc.get_activation_tables = _orig
        nc.m = get_hw_module(nc.m)
        _CACHE["nc"] = nc
    return _CACHE["nc"]


def _host_params(centers, covs_inv_sqrt, weights, threshold):
    """Per-cluster parameter rows (B) scaled for fp8, plus exp(-C)."""
    S = covs_inv_sqrt.astype(np.float64)
    w = np.abs(weights.astype(np.float64))
    cp = w / (w.sum() + 1e-30)
    A = np.einsum("kde,kfe->kdf", S, S)
    _, logdetS = np.linalg.slogdet(S)
    logcoef = np.log(np.maximum(cp, 1e-300)) + logdetS
    cen = centers.astype(np.float64)
    m = np.einsum("kde,ke->kd", A, cen)
    t_cAc = np.einsum("kd,kd->k", m, cen)
    thr = float(threshold[0])
    C = 4.0 - (logcoef.max() - thr)
    bias = logcoef - 0.5 * t_cAc - thr + C

    Brows = np.zeros((NROWS, K), np.float64)
    r = 0
    for o in range(1, 16):                      # 480 off-diag product rows
        for i in range(32):
            Brows[r] = -A[:, i, (i + o) % 32] * SCL_P
            r += 1
    for i in range(16):                         # 16 distance-16 pairs
        Brows[r] = -A[:, i, i + 16] * SCL_P
        r += 1
    for i in range(32):                         # squares
        Brows[r] = -0.5 * A[:, i, i] * SCL_D
        r += 1
    for i in range(32):                         # linear
        Brows[r] = m[:, i] * SCL_L
        r += 1
    f8 = ml_dtypes.float8_e4m3
    b0 = bias.astype(f8).astype(np.float64)
    b1 = (bias - b0).astype(f8).astype(np.float64)
    Brows[r] = b0
    Brows[r + 1] = b1
    Brows[r + 2] = bias - b0 - b1
    # row r+3 stays zero (pad)
    return Brows.astype(np.float32), np.float64(C)


def _host_lift(pts):
    """fp8-ready lift planes [NROWS, npts] for one core's points [npts, 32]."""
    X = np.ascontiguousarray(pts.T)             # [32, npts]
    npts = X.shape[1]
    L = np.empty((NROWS, npts), np.float32)
    r = 0
    inv_p = np.float32(1.0 / SCL_P)
    for o in range(1, 16):
        L[r:r + 32] = X * np.roll(X, -o, axis=0) * inv_p
        r += 32
    L[r:r + 16] = X[:16] * X[16:] * inv_p
    r += 16
    L[r:r + 32] = X * X * np.float32(1.0 / SCL_D)
    r += 32
    L[r:r + 32] = X * np.float32(1.0 / SCL_L)
    r += 32
    L[r:r + 2] = 1.0
    L[r + 2] = 1.0
    L[r + 3] = 0.0
    return L


def kernel(points, centers, covs_inv_sqrt, weights, threshold):
    points = np.asarray(points, dtype=np.float32)
    Brows, C = _host_params(np.asarray(centers),
                            np.asarray(covs_inv_sqrt),
                            np.asarray(weights), np.asarray(threshold))
    f8 = ml_dtypes.float8_e4m3
    B8 = Brows.astype(f8)
    bab = np.ascontiguousarray(
        B8[:512].reshape(4, 128, K).transpose(1, 0, 2))
    bc = np.ascontiguousarray(
        B8[512:].reshape(2, CROWS, K).transpose(1, 0, 2))
    in_maps = []
    for r in range(NCORES):
        L8 = _host_lift(points[r * NC_PTS:(r + 1) * NC_PTS]).astype(f8)
        lab = np.ascontiguousarray(
            L8[:512].reshape(4, 128, NC_PTS).transpose(1, 0, 2))
        lc = np.ascontiguousarray(
            L8[512:].reshape(2, CROWS, NC_PTS).transpose(1, 0, 2))
        in_maps.append({"lab": lab, "lc": lc, "bab": bab, "bc": bc})

    nc = _get_module()
    res = bass_utils.run_bass_kernel_spmd(nc, in_maps,
                                          core_ids=list(range(NCORES)))
    sv = np.concatenate([res.results[r]["out"].T.reshape(-1)
                         for r in range(NCORES)])
    ll = np.log(sv.astype(np.float64)) - C
    return ll.reshape(N, 1).astype(np.float32)


# revision 44
# speedup vs baseline: 1.0209x; 1.0035x over previous
"""Gaussian-mixture log-likelihood kernel for 8 Trainium2 NeuronCores.

Math: ll_i = logsumexp_j( -0.5 x_i^T A_j x_i + x_i^T m_j + bias_j ) - C
with A_j = S_j S_j^T.  The quadratic form is a single contraction of 564
"lift" rows per point against a [564, K] parameter matrix: 496 unique
symmetric-pair products, 32 squares, 32 linear rows, 3 bias rows (split so
fp8 quantization of the bias is exact to ~1e-3) and 1 zero pad.  Lift rows
and parameters are stored in fp8-e4m3 with per-row-type power-of-two scales
folded oppositely into the two factors, so the PE runs DoubleRow matmuls
(256-row contraction per instruction, 2x column rate).

Layout is K-on-partitions: the parameter matrix is the stationary operand
and scores land as [K-half, points] in PSUM, 6 matmuls per 256-point chunk.
One [128, 2048] Exp turns a 4-chunk PSUM slab into f16 likelihoods; the sum
over K is then a second matmul pass (e-tile stationary x a ones column,
output free size 1), which costs the PE almost nothing and keeps the sum in
f32.  A global shift C (folded into the bias rows) makes exp() safe without
a per-point max; one Ln + bias-add finishes all 16384 points per core.

The lift planes are packed host-side (cheap: O(N D^2) vs the device's
O(N K D^2) contraction) and shipped as fp8, which also removes all
shuffle/multiply traffic from DVE/Pool.

Sharding: data-parallel over points, 16384 points/core; K-sized parameters
are replicated (precomputed on host in float64 -- tiny vs the N*K work).
"""

import sys

sys.path.insert(0, "/opt/trn_rl_repo")

import numpy as np
import ml_dtypes

import concourse.bass as bass
import concourse.bacc as bacc
import concourse.mybir as mybir
from concourse import bass_utils
from concourse.bass_interp import get_hw_module
from concourse.tile import TileContext

N, K, D = 131072, 256, 32
NCORES = 8
NC_PTS = N // NCORES            # 16384 points per core
NTILES = NC_PTS // 128          # 128 output columns
DSLAB = 1024                    # points per DMA slab
NDMA = NC_PTS // DSLAB          # 16
PSLAB = 1024                    # points per PSUM slab (4 chunks x 2 K-halves)
CHUNK = 256                     # points per matmul chunk
F32 = mybir.dt.float32
F16 = mybir.dt.float16
F8 = mybir.dt.float8e4

NROWS = 564                     # 512 (chunks A,B) + 52 (chunk C)
CROWS = 26                      # chunk C pair rows

SCL_P = 16.0                    # off-diagonal product rows
SCL_D = 4.0                     # diagonal (square) rows
SCL_L = 4.0                     # linear rows

_CACHE = {}


def _build(nc):
    lab = nc.dram_tensor("lab", [128, 4, NC_PTS], F8, kind="ExternalInput").ap()
    lc = nc.dram_tensor("lc", [CROWS, 2, NC_PTS], F8, kind="ExternalInput").ap()
    bab = nc.dram_tensor("bab", [128, 4, K], F8, kind="ExternalInput").ap()
    bc = nc.dram_tensor("bc", [CROWS, 2, K], F8, kind="ExternalInput").ap()
    out = nc.dram_tensor("out", [128, NTILES], F32, kind="ExternalOutput").ap()

    DR = mybir.MatmulPerfMode.DoubleRow

    with TileContext(nc) as tc:
        with (
            tc.tile_pool(name="par", bufs=1) as par_pool,
            tc.tile_pool(name="src", bufs=16) as src_pool,
            tc.tile_pool(name="eps", bufs=16) as eps_pool,
            tc.tile_pool(name="acc", bufs=1) as acc_pool,
            tc.tile_pool(name="psum", bufs=2, space="PSUM") as psum_pool,
        ):
            # split param issue across the SP and ACT DGE queues so the lift
            # stream and the (tiny) params overlap at startup.
            bab_t = par_pool.tile([128, 4, K], F8, tag="bab")
            bc_t = par_pool.tile([CROWS, 2, K], F8, tag="bc")
            ones = par_pool.tile([128, 1], F16, tag="ones")
            warm = par_pool.tile([128, 512], F16, tag="warm")
            nc.sync.dma_start(out=bab_t[:, :, :], in_=bab[:, :, :])
            nc.scalar.dma_start(out=bc_t[:, :, :], in_=bc[:, :, :])
            nc.gpsimd.memset(ones[:, :], 1.0)
            nc.gpsimd.memset(warm[:, :], 0.0)

            s_all = acc_pool.tile([128, NTILES], F32, tag="s_all")

            # ramp the PE p-state on junk during the initial DMA window so
            # the first real matmuls run at (nearly) full clock
            ps_warm = psum_pool.tile([128, 4, 2, CHUNK], F32, tag="ps")
            for _ in range(3):
                nc.tensor.matmul(out=ps_warm[0:1, 0, :, :],
                                 lhsT=warm[:, 0:1], rhs=warm[:, :],
                                 start=True, stop=True)

            def emit_sums(ps, e_t, col, ntiles):
                # sum over K: e-tile stationary x ones column -> [128, 1] per
                # point-tile, accumulated into columns of bank 0 of the (now
                # dead) score psum.  Emitted one slab late so the in-order PE
                # stream never stalls on the Exp of its own slab.
                for u in range(ntiles):
                    c, w = u // 2, u % 2
                    sl = slice(128 * w, 128 * (w + 1))
                    for h in range(2):
                        nc.tensor.matmul(out=ps[:, 0, 0, u:u + 1],
                                         lhsT=e_t[:, c, h, sl],
                                         rhs=ones[:, :],
                                         start=(u == 0 and h == 0),
                                         stop=(u == ntiles - 1 and h == 1))
                nc.vector.tensor_copy(out=s_all[:, col:col + ntiles],
                                      in_=ps[:, 0, 0, 0:ntiles])

            # 512-point slabs at both ends shorten the pipeline fill (first
            # Exp waits on half the data) and drain; 1024-point slabs in the
            # middle amortize the ACT per-instruction overhead.
            sizes = [1024] * 16
            pending = None
            lo = 0
            for npts in sizes:
                dlo = lo - lo % DSLAB
                if lo == dlo:  # first slab touching this DMA tile loads it
                    lab_t = src_pool.tile([128, 4, DSLAB], F8, tag="lab")
                    lc_t = src_pool.tile([CROWS, 2, DSLAB], F8, tag="lc")
                    if lo == 0:
                        nc.sync.dma_start(out=lab_t[:, :, 0:512],
                                          in_=lab[:, :, 0:512])
                        nc.gpsimd.dma_start(out=lc_t[:, :, :],
                                            in_=lc[:, :, 0:DSLAB])
                        nc.sync.dma_start(out=lab_t[:, :, 512:DSLAB],
                                          in_=lab[:, :, 512:DSLAB])
                    else:
                        nc.sync.dma_start(out=lab_t[:, :, :],
                                          in_=lab[:, :, dlo:dlo + DSLAB])
                        nc.gpsimd.dma_start(out=lc_t[:, :, :],
                                            in_=lc[:, :, dlo:dlo + DSLAB])

                nch = npts // CHUNK
                ps = psum_pool.tile([128, 4, 2, CHUNK], F32, tag="ps")
                for c in range(nch):
                    cs = slice(lo - dlo + c * CHUNK, lo - dlo + (c + 1) * CHUNK)
                    for h in range(2):
                        ks = slice(128 * h, 128 * (h + 1))
                        nc.tensor.matmul(out=ps[:, c, h, :],
                                         lhsT=bab_t[:, 0:2, ks],
                                         rhs=lab_t[:, 0:2, cs],
                                         start=(h == 0), stop=False,
                                         perf_mode=DR)
                        nc.tensor.matmul(out=ps[:, c, h, :],
                                         lhsT=bab_t[:, 2:4, ks],
                                         rhs=lab_t[:, 2:4, cs],
                                         start=False, stop=False,
                                         perf_mode=DR)
                        nc.tensor.matmul(out=ps[:, c, h, :],
                                         lhsT=bc_t[:, :, ks],
                                         rhs=lc_t[:, :, cs],
                                         start=False, stop=(h == 1),
                                         perf_mode=DR)
                e_t = eps_pool.tile([128, 4, 2, CHUNK], F16, tag="e")
                nc.scalar.activation(out=e_t[:, 0:nch, :, :],
                                     in_=ps[:, 0:nch, :, :],
                                     func=mybir.ActivationFunctionType.Exp)
                if pending is not None:
                    emit_sums(*pending)
                    pc = pending[2] + pending[3]
                    if pc == 64:
                        nc.scalar.dma_start(out=out[:, 0:64],
                                            in_=s_all[:, 0:64])
                    elif pc == 112:
                        nc.scalar.dma_start(out=out[:, 64:112],
                                            in_=s_all[:, 64:112])
                pending = (ps, e_t, lo // 128, npts // 128)
                lo += npts
            emit_sums(*pending)
            # the final ln(s) - C runs on host; ship the sums
            nc.sync.dma_start(out=out[:, 112:], in_=s_all[:, 112:])
    return nc


def _get_module():
    if "nc" not in _CACHE:
        nc = bacc.Bacc("TRN2", target_bir_lowering=False, debug=False,
                       num_devices=NCORES)
        _build(nc)
        # During the act-table placement pass only, hide every func set except
        # natural_log_exp_and_others (its real act_info index is preserved),
        # so Exp and Ln share one table load instead of reloading before the
        # final Ln.  Restored immediately -- runtime sees the true tables.
        import concourse.hw_specs as _hw_specs
        _orig = _hw_specs.get_activation_tables

        def _patched(arch):
            return {name: (funcs if name == "natural_log_exp_and_others"
                           else set())
                    for name, funcs in _orig(arch).items()}

        if False:  # runtime rejects set 6 (NRT_EXEC_UNIT_UNRECOVERABLE)
            _hw_specs.get_activation_tables = _patched
            bacc.get_activation_tables = _patched
        try:
            nc.compile()
        finally:
            _hw_specs.get_activation_tables = _orig
            bacc.get_activation_tables = _orig
        nc.m = get_hw_module(nc.m)
        _CACHE["nc"] = nc
    return _CACHE["nc"]


def _host_params(centers, covs_inv_sqrt, weights, threshold):
    """Per-cluster parameter rows (B) scaled for fp8, plus exp(-C)."""
    S = covs_inv_sqrt.astype(np.float64)
    w = np.abs(weights.astype(np.float64))
    cp = w / (w.sum() + 1e-30)
    A = np.einsum("kde,kfe->kdf", S, S)
    _, logdetS = np.linalg.slogdet(S)
    logcoef = np.log(np.maximum(cp, 1e-300)) + logdetS
    cen = centers.astype(np.float64)
    m = np.einsum("kde,ke->kd", A, cen)
    t_cAc = np.einsum("kd,kd->k", m, cen)
    thr = float(threshold[0])
    C = 4.0 - (logcoef.max() - thr)
    bias = logcoef - 0.5 * t_cAc - thr + C

    Brows = np.zeros((NROWS, K), np.float64)
    r = 0
    for o in range(1, 16):                      # 480 off-diag product rows
        for i in range(32):
            Brows[r] = -A[:, i, (i + o) % 32] * SCL_P
            r += 1
    for i in range(16):                         # 16 distance-16 pairs
        Brows[r] = -A[:, i, i + 16] * SCL_P
        r += 1
    for i in range(32):                         # squares
        Brows[r] = -0.5 * A[:, i, i] * SCL_D
        r += 1
    for i in range(32):                         # linear
        Brows[r] = m[:, i] * SCL_L
        r += 1
    f8 = ml_dtypes.float8_e4m3
    b0 = bias.astype(f8).astype(np.float64)
    b1 = (bias - b0).astype(f8).astype(np.float64)
    Brows[r] = b0
    Brows[r + 1] = b1
    Brows[r + 2] = bias - b0 - b1
    # row r+3 stays zero (pad)
    return Brows.astype(np.float32), np.float64(C)


def _host_lift(pts):
    """fp8-ready lift planes [NROWS, npts] for one core's points [npts, 32]."""
    X = np.ascontiguousarray(pts.T)             # [32, npts]
    npts = X.shape[1]
    L = np.empty((NROWS, npts), np.float32)
    r = 0
    inv_p = np.float32(1.0 / SCL_P)
    for o in range(1, 16):
        L[r:r + 32] = X * np.roll(X, -o, axis=0) * inv_p
        r += 32
    L[r:r + 16] = X[:16] * X[16:] * inv_p
    r += 16
    L[r:r + 32] = X * X * np.float32(1.0 / SCL_D)
    r += 32
    L[r:r + 32] = X * np.float32(1.0 / SCL_L)
    r += 32
    L[r:r + 2] = 1.0
    L[r + 2] = 1.0
    L[r + 3] = 0.0
    return L


def kernel(points, centers, covs_inv_sqrt, weights, threshold):
    points = np.asarray(points, dtype=np.float32)
    Brows, C = _host_params(np.asarray(centers),
                            np.asarray(covs_inv_sqrt),
                            np.asarray(weights), np.asarray(threshold))
    f8 = ml_dtypes.float8_e4m3
    B8 = Brows.astype(f8)
    bab = np.ascontiguousarray(
        B8[:512].reshape(4, 128, K).transpose(1, 0, 2))
    bc = np.ascontiguousarray(
        B8[512:].reshape(2, CROWS, K).transpose(1, 0, 2))
    in_maps = []
    for r in range(NCORES):
        L8 = _host_lift(points[r * NC_PTS:(r + 1) * NC_PTS]).astype(f8)
        lab = np.ascontiguousarray(
            L8[:512].reshape(4, 128, NC_PTS).transpose(1, 0, 2))
        lc = np.ascontiguousarray(
            L8[512:].reshape(2, CROWS, NC_PTS).transpose(1, 0, 2))
        in_maps.append({"lab": lab, "lc": lc, "bab": bab, "bc": bc})

    nc = _get_module()
    res = bass_utils.run_bass_kernel_spmd(nc, in_maps,
                                          core_ids=list(range(NCORES)))
    sv = np.concatenate([res.results[r]["out"].T.reshape(-1)
                         for r in range(NCORES)])
    ll = np.log(sv.astype(np.float64)) - C
    return ll.reshape(N, 1).astype(np.float32)
